# revision 3
# baseline (speedup 1.0000x reference)
"""HOGRUCell (RK4-integrated GRU-like ODE cell) Trainium2 Bass kernel.

Strategy: pure data parallel over 8 NeuronCores (batch sharded). All compute
in a transposed [features, batch] layout so every linear-layer contraction is
already on the partition dim and biases become per-partition ACT scalars.
Matmuls in bf16 (PE-bound problem: ~72 GFLOP/core), state updates fp32 at the
final combine.
"""

import numpy as np
import ml_dtypes

B, KI, K, KO = 16384, 256, 1024, 256
NCORES = 8
BC = B // NCORES          # 2048 batch rows per core
TN = 512                  # batch tile (free dim per matmul)
NT = BC // TN             # 4 tiles per core
KC = K // 128             # 8 chunks of the 1024 state dim
KIC = KI // 128           # 2 chunks of input dim
GM = 2 * K // 128         # 16 chunks of the 2048 gate dim
LXM = 3 * K // 128        # 24 chunks of the 3072 lx dim
YM = KO // 128            # 2 chunks of output dim

_BF16 = ml_dtypes.bfloat16

_cache = {}


def _pack_w(w, kc):
    """[out_f, in_f] weight -> lhsT packed [128, kc, out_f] (k-chunk on free)."""
    in_f, out_f = w.shape[1], w.shape[0]
    assert in_f == kc * 128
    return np.ascontiguousarray(
        w.T.reshape(kc, 128, out_f).transpose(1, 0, 2)
    ).astype(_BF16)


def _pack_b(b):
    """[F] bias -> [128, F//128] per-partition scalars."""
    return np.ascontiguousarray(b.reshape(-1, 128).T).astype(np.float32)


def _pack_act(a, kc, dtype):
    """[rows, F] activations -> [128, kc, rows] transposed+chunked."""
    rows = a.shape[0]
    return np.ascontiguousarray(
        a.T.reshape(kc, 128, rows).transpose(1, 0, 2)
    ).astype(dtype)


def _unpack_act(a, kc, rows):
    """[128, kc, rows] -> [rows, kc*128]."""
    return np.ascontiguousarray(a.transpose(1, 0, 2).reshape(kc * 128, rows).T)


def _build(dt_val: float):
    import concourse.bacc as bacc
    import concourse.mybir as mybir
    import concourse.tile as tile
    from concourse.bass import ts

    f32 = mybir.dt.float32
    bf16 = mybir.dt.bfloat16
    AF = mybir.ActivationFunctionType
    ALU = mybir.AluOpType

    nc = bacc.Bacc("TRN2", target_bir_lowering=False, debug=False)

    xt = nc.dram_tensor("xt", [128, KIC, BC], bf16, kind="ExternalInput")
    st = nc.dram_tensor("st", [128, KC, BC], f32, kind="ExternalInput")
    w_in = nc.dram_tensor("w_in", [128, KIC, K], bf16, kind="ExternalInput")
    w_lx = nc.dram_tensor("w_lx", [128, KC, 3 * K], bf16, kind="ExternalInput")
    w_hg = nc.dram_tensor("w_hg", [128, KC, 2 * K], bf16, kind="ExternalInput")
    w_hl = nc.dram_tensor("w_hl", [128, KC, K], bf16, kind="ExternalInput")
    w_y1 = nc.dram_tensor("w_y1", [128, KC, K], bf16, kind="ExternalInput")
    w_y2 = nc.dram_tensor("w_y2", [128, KC, KO], bf16, kind="ExternalInput")
    b_in = nc.dram_tensor("b_in", [128, KC], f32, kind="ExternalInput")
    b_hg = nc.dram_tensor("b_hg", [128, GM], f32, kind="ExternalInput")
    b_hl = nc.dram_tensor("b_hl", [128, KC], f32, kind="ExternalInput")
    yt = nc.dram_tensor("yt", [128, YM, BC], f32, kind="ExternalOutput")
    snt = nc.dram_tensor("snt", [128, KC, BC], f32, kind="ExternalOutput")

    dma = nc.sync.dma_start

    with tile.TileContext(nc) as tc:
        with (
            tc.tile_pool(name="wres", bufs=1) as wres,
            tc.tile_pool(name="wstream", bufs=4) as wstream,
            tc.tile_pool(name="big", bufs=1) as big,
            tc.tile_pool(name="dbl", bufs=1) as dbl,
            tc.tile_pool(name="chunk", bufs=2) as chunk,
            tc.tile_pool(name="ppg", bufs=3, space="PSUM") as ppg,
            tc.tile_pool(name="ppl", bufs=2, space="PSUM") as ppl,
            tc.tile_pool(name="pp1", bufs=2, space="PSUM") as pp1,
        ):
            # resident weights
            w_in_sb = wres.tile([128, KIC, K], bf16)
            dma(w_in_sb[:], w_in[:])
            w_hg_sb = wres.tile([128, KC, 2 * K], bf16)
            dma(w_hg_sb[:], w_hg[:])
            w_hl_sb = wres.tile([128, KC, K], bf16)
            dma(w_hl_sb[:], w_hl[:])
            b_in_sb = wres.tile([128, KC], f32)
            dma(b_in_sb[:], b_in[:])
            b_hg_sb = wres.tile([128, GM], f32)
            dma(b_hg_sb[:], b_hg[:])
            b_hl_sb = wres.tile([128, KC], f32)
            dma(b_hl_sb[:], b_hl[:])

            for t in range(NT):
                cols = slice(t * TN, (t + 1) * TN)

                xt_sb = dbl.tile([128, KIC, TN], bf16, tag="xt")
                dma(xt_sb[:], xt[:, :, cols])
                h0f = dbl.tile([128, KC, TN], f32, tag="h0f")
                dma(h0f[:], st[:, :, cols])
                h0b = dbl.tile([128, KC, TN], bf16, tag="h0b")
                nc.gpsimd.tensor_copy(h0b[:], h0f[:])

                # x0 = tanh(W_in @ x + b_in)
                x0 = dbl.tile([128, KC, TN], bf16, tag="x0", bufs=2)
                for m in range(KC):
                    ps = pp1.tile([128, TN], f32, tag="ps1")
                    for k in range(KIC):
                        nc.tensor.matmul(
                            ps[:], w_in_sb[:, k, ts(m, 128)], xt_sb[:, k, :],
                            start=(k == 0), stop=(k == KIC - 1),
                        )
                    nc.scalar.activation(
                        x0[:, m, :], ps[:], AF.Tanh, bias=b_in_sb[:, m : m + 1]
                    )

                # lx = W_lx @ x0   (streamed weights, bf16 copy to SBUF)
                lx = big.tile([128, LXM, TN], bf16, tag="lx")
                for m in range(LXM):
                    wc = wstream.tile([128, KC, 128], bf16, tag="wst")
                    dma(wc[:], w_lx[:, :, ts(m, 128)])
                    ps = pp1.tile([128, TN], f32, tag="ps1")
                    for k in range(KC):
                        nc.tensor.matmul(
                            ps[:], wc[:, k, :], x0[:, k, :],
                            start=(k == 0), stop=(k == KC - 1),
                        )
                    nc.scalar.copy(lx[:, m, :], ps[:])

                # RK4 stages
                ksum = big.tile([128, KC, TN], bf16, tag="ksum")
                h_cur = h0b
                for s in (1, 2, 3, 4):
                    zz = dbl.tile([128, KC, TN], bf16, tag="zz")
                    rr = dbl.tile([128, KC, TN], bf16, tag="rr")
                    for m in range(GM):
                        ps = ppg.tile([128, TN], f32, tag="psg")
                        for k in range(KC):
                            nc.tensor.matmul(
                                ps[:], w_hg_sb[:, k, ts(m, 128)], h_cur[:, k, :],
                                start=(k == 0), stop=(k == KC - 1),
                            )
                        garg = chunk.tile([128, TN], bf16, tag="arg")
                        nc.vector.tensor_add(garg[:], ps[:], lx[:, m, :])
                        dst = zz if m < KC else rr
                        nc.scalar.activation(
                            dst[:, m % KC, :], garg[:], AF.Sigmoid,
                            bias=b_hg_sb[:, m : m + 1],
                        )
                    rh = dbl.tile([128, KC, TN], bf16, tag="rh")
                    for m in range(KC):
                        nc.vector.tensor_mul(rh[:, m, :], rr[:, m, :], h_cur[:, m, :])

                    h_next = (
                        dbl.tile([128, KC, TN], bf16, tag="hn", name="hn", bufs=2)
                        if s < 4 else None
                    )
                    for m in range(KC):
                        ps = ppl.tile([128, TN], f32, tag="psl")
                        for k in range(KC):
                            nc.tensor.matmul(
                                ps[:], w_hl_sb[:, k, ts(m, 128)], rh[:, k, :],
                                start=(k == 0), stop=(k == KC - 1),
                            )
                        larg = chunk.tile([128, TN], bf16, tag="arg")
                        nc.vector.tensor_add(larg[:], ps[:], lx[:, 2 * KC + m, :])
                        tt = chunk.tile([128, TN], bf16, tag="tt")
                        nc.scalar.activation(
                            tt[:], larg[:], AF.Tanh, bias=b_hl_sb[:, m : m + 1]
                        )
                        dd = chunk.tile([128, TN], bf16, tag="dd")
                        nc.vector.tensor_sub(dd[:], tt[:], h_cur[:, m, :])
                        if s == 1:
                            nc.vector.tensor_mul(ksum[:, m, :], zz[:, m, :], dd[:])
                            nc.vector.scalar_tensor_tensor(
                                h_next[:, m, :], ksum[:, m, :], dt_val / 2,
                                h0b[:, m, :], ALU.mult, ALU.add,
                            )
                        else:
                            kk = chunk.tile([128, TN], bf16, tag="kk")
                            nc.vector.tensor_mul(kk[:], zz[:, m, :], dd[:])
                            w_s = {2: 2.0, 3: 2.0, 4: 1.0}[s]
                            nc.vector.scalar_tensor_tensor(
                                ksum[:, m, :], kk[:], w_s, ksum[:, m, :],
                                ALU.mult, ALU.add,
                            )
                            if s < 4:
                                c_s = dt_val / 2 if s == 2 else dt_val
                                nc.vector.scalar_tensor_tensor(
                                    h_next[:, m, :], kk[:], c_s, h0b[:, m, :],
                                    ALU.mult, ALU.add,
                                )
                    if s < 4:
                        h_cur = h_next

                # state_new = h0 + dt/6 * ksum  (fp32, in place over h0f)
                for m in range(KC):
                    nc.vector.scalar_tensor_tensor(
                        h0f[:, m, :], ksum[:, m, :], dt_val / 6, h0f[:, m, :],
                        ALU.mult, ALU.add,
                    )
                dma(snt[:, :, cols], h0f[:])

                # y = W_y2 @ tanh(W_y1 @ state_new)
                snb = dbl.tile([128, KC, TN], bf16, tag="x0", bufs=2)
                nc.gpsimd.tensor_copy(snb[:], h0f[:])
                gy = dbl.tile([128, KC, TN], bf16, tag="zz")
                for m in range(KC):
                    wc = wstream.tile([128, KC, 128], bf16, tag="wst")
                    dma(wc[:], w_y1[:, :, ts(m, 128)])
                    ps = pp1.tile([128, TN], f32, tag="ps1")
                    for k in range(KC):
                        nc.tensor.matmul(
                            ps[:], wc[:, k, :], snb[:, k, :],
                            start=(k == 0), stop=(k == KC - 1),
                        )
                    nc.scalar.activation(gy[:, m, :], ps[:], AF.Tanh)
                y_sb = dbl.tile([128, YM, TN], f32, tag="y")
                for m in range(YM):
                    wc = wstream.tile([128, KC, 128], bf16, tag="wst")
                    dma(wc[:], w_y2[:, :, ts(m, 128)])
                    ps = pp1.tile([128, TN], f32, tag="ps1")
                    for k in range(KC):
                        nc.tensor.matmul(
                            ps[:], wc[:, k, :], gy[:, k, :],
                            start=(k == 0), stop=(k == KC - 1),
                        )
                    nc.scalar.copy(y_sb[:, m, :], ps[:])
                dma(yt[:, :, cols], y_sb[:])

    nc.compile()
    return nc


def _get_nc(dt_val: float):
    key = round(float(dt_val), 9)
    if key not in _cache:
        _cache[key] = _build(key)
    return _cache[key]


def kernel(x, state, dt, W_in, b_in, W_lx, W_hg, b_hg, W_hl, b_hl, W_y1, W_y2,
           trace=False):
    from concourse.bass_utils import run_bass_kernel_spmd

    dt_val = float(np.asarray(dt).reshape(-1)[0])
    nc = _get_nc(dt_val)

    x = np.asarray(x, np.float32)
    state = np.asarray(state, np.float32)
    weights = {
        "w_in": _pack_w(np.asarray(W_in, np.float32), KIC),
        "w_lx": _pack_w(np.asarray(W_lx, np.float32), KC),
        "w_hg": _pack_w(np.asarray(W_hg, np.float32), KC),
        "w_hl": _pack_w(np.asarray(W_hl, np.float32), KC),
        "w_y1": _pack_w(np.asarray(W_y1, np.float32), KC),
        "w_y2": _pack_w(np.asarray(W_y2, np.float32), KC),
        "b_in": _pack_b(np.asarray(b_in, np.float32)),
        "b_hg": _pack_b(np.asarray(b_hg, np.float32)),
        "b_hl": _pack_b(np.asarray(b_hl, np.float32)),
    }
    in_maps = []
    for c in range(NCORES):
        rows = slice(c * BC, (c + 1) * BC)
        m = dict(weights)
        m["xt"] = _pack_act(x[rows], KIC, _BF16)
        m["st"] = _pack_act(state[rows], KC, np.float32)
        in_maps.append(m)

    res = run_bass_kernel_spmd(nc, in_maps, core_ids=list(range(NCORES)),
                               trace=trace)

    y = np.empty((B, KO), np.float32)
    sn = np.empty((B, K), np.float32)
    for c in range(NCORES):
        rows = slice(c * BC, (c + 1) * BC)
        y[rows] = _unpack_act(res.results[c]["yt"], YM, BC)
        sn[rows] = _unpack_act(res.results[c]["snt"], KC, BC)
    if trace:
        kernel.last_results = res
    return y, sn


# revision 5
# speedup vs baseline: 1.0661x; 1.0661x over previous
"""HOGRUCell (RK4-integrated GRU-like ODE cell) Trainium2 Bass kernel.

Strategy: pure data parallel over 8 NeuronCores (batch sharded). All compute
in a transposed [features, batch] layout so every linear-layer contraction is
already on the partition dim and biases become per-partition ACT scalars.
Matmuls in bf16 (PE-bound problem: ~72 GFLOP/core), state updates fp32 at the
final combine.
"""

import numpy as np
import ml_dtypes

B, KI, K, KO = 16384, 256, 1024, 256
NCORES = 8
BC = B // NCORES          # 2048 batch rows per core
TN = 512                  # batch tile (free dim per matmul)
NT = BC // TN             # 4 tiles per core
KC = K // 128             # 8 chunks of the 1024 state dim
KIC = KI // 128           # 2 chunks of input dim
GM = 2 * K // 128         # 16 chunks of the 2048 gate dim
LXM = 3 * K // 128        # 24 chunks of the 3072 lx dim
YM = KO // 128            # 2 chunks of output dim

_BF16 = ml_dtypes.bfloat16

_cache = {}


def _pack_w(w, kc):
    """[out_f, in_f] weight -> lhsT packed [128, kc, out_f] (k-chunk on free)."""
    in_f, out_f = w.shape[1], w.shape[0]
    assert in_f == kc * 128
    return np.ascontiguousarray(
        w.T.reshape(kc, 128, out_f).transpose(1, 0, 2)
    ).astype(_BF16)


def _pack_b(b):
    """[F] bias -> [128, F//128] per-partition scalars."""
    return np.ascontiguousarray(b.reshape(-1, 128).T).astype(np.float32)


def _pack_act(a, kc, dtype):
    """[rows, F] activations -> [128, kc, rows] transposed+chunked."""
    rows = a.shape[0]
    return np.ascontiguousarray(
        a.T.reshape(kc, 128, rows).transpose(1, 0, 2)
    ).astype(dtype)


def _unpack_act(a, kc, rows):
    """[128, kc, rows] -> [rows, kc*128]."""
    return np.ascontiguousarray(a.transpose(1, 0, 2).reshape(kc * 128, rows).T)


def _build(dt_val: float):
    import concourse.bacc as bacc
    import concourse.mybir as mybir
    import concourse.tile as tile
    from concourse.bass import ts

    f32 = mybir.dt.float32
    bf16 = mybir.dt.bfloat16
    AF = mybir.ActivationFunctionType
    ALU = mybir.AluOpType

    nc = bacc.Bacc("TRN2", target_bir_lowering=False, debug=False)

    xt = nc.dram_tensor("xt", [128, KIC, BC], bf16, kind="ExternalInput")
    st = nc.dram_tensor("st", [128, KC, BC], f32, kind="ExternalInput")
    w_in = nc.dram_tensor("w_in", [128, KIC, K], bf16, kind="ExternalInput")
    w_lx = nc.dram_tensor("w_lx", [128, KC, 3 * K], bf16, kind="ExternalInput")
    w_hg = nc.dram_tensor("w_hg", [128, KC, 2 * K], bf16, kind="ExternalInput")
    w_hl = nc.dram_tensor("w_hl", [128, KC, K], bf16, kind="ExternalInput")
    w_y1 = nc.dram_tensor("w_y1", [128, KC, K], bf16, kind="ExternalInput")
    w_y2 = nc.dram_tensor("w_y2", [128, KC, KO], bf16, kind="ExternalInput")
    b_in = nc.dram_tensor("b_in", [128, KC], f32, kind="ExternalInput")
    b_hg = nc.dram_tensor("b_hg", [128, GM], f32, kind="ExternalInput")
    b_hl = nc.dram_tensor("b_hl", [128, KC], f32, kind="ExternalInput")
    yt = nc.dram_tensor("yt", [128, YM, BC], f32, kind="ExternalOutput")
    snt = nc.dram_tensor("snt", [128, KC, BC], f32, kind="ExternalOutput")

    dma = nc.sync.dma_start

    with tile.TileContext(nc) as tc:
        with (
            tc.tile_pool(name="wres", bufs=1) as wres,
            tc.tile_pool(name="wstream", bufs=4) as wstream,
            tc.tile_pool(name="big", bufs=1) as big,
            tc.tile_pool(name="dbl", bufs=1) as dbl,
            tc.tile_pool(name="chunk", bufs=2) as chunk,
            tc.tile_pool(name="ppg", bufs=3, space="PSUM") as ppg,
            tc.tile_pool(name="ppl", bufs=2, space="PSUM") as ppl,
            tc.tile_pool(name="pp1", bufs=2, space="PSUM") as pp1,
        ):
            # resident weights
            w_in_sb = wres.tile([128, KIC, K], bf16)
            dma(w_in_sb[:], w_in[:])
            w_hg_sb = wres.tile([128, KC, 2 * K], bf16)
            for k in range(KC):
                dma(w_hg_sb[:, k, :], w_hg[:, k, :])
            w_hl_sb = wres.tile([128, KC, K], bf16)
            for k in range(KC):
                dma(w_hl_sb[:, k, :], w_hl[:, k, :])
            b_in_sb = wres.tile([128, KC], f32)
            dma(b_in_sb[:], b_in[:])
            b_hg_sb = wres.tile([128, GM], f32)
            dma(b_hg_sb[:], b_hg[:])
            b_hl_sb = wres.tile([128, KC], f32)
            dma(b_hl_sb[:], b_hl[:])

            def emit_head(t):
                cols = slice(t * TN, (t + 1) * TN)
                xt_sb = dbl.tile([128, KIC, TN], bf16, tag="xt", name="xt_sb")
                dma(xt_sb[:], xt[:, :, cols])
                h0f = dbl.tile([128, KC, TN], f32, tag="h0f", name="h0f")
                dma(h0f[:], st[:, :, cols])
                h0b = dbl.tile([128, KC, TN], bf16, tag="h0b", name="h0b")
                for m in range(KC):
                    nc.vector.tensor_copy(h0b[:, m, :], h0f[:, m, :])

                # x0 = tanh(W_in @ x + b_in)
                x0 = dbl.tile([128, KC, TN], bf16, tag="x0", bufs=2, name="x0")
                for m in range(KC):
                    ps = pp1.tile([128, TN], f32, tag="ps1")
                    for k in range(KIC):
                        nc.tensor.matmul(
                            ps[:], w_in_sb[:, k, ts(m, 128)], xt_sb[:, k, :],
                            start=(k == 0), stop=(k == KIC - 1),
                        )
                    nc.scalar.activation(
                        x0[:, m, :], ps[:], AF.Tanh, bias=b_in_sb[:, m : m + 1]
                    )

                # lx = W_lx @ x0   (streamed weights, bf16 copy to SBUF)
                lx = big.tile([128, LXM, TN], bf16, tag="lx", name="lx")
                for m in range(LXM):
                    wc = wstream.tile([128, KC, 128], bf16, tag="wst")
                    dma(wc[:], w_lx[:, :, ts(m, 128)])
                    ps = pp1.tile([128, TN], f32, tag="ps1")
                    for k in range(KC):
                        nc.tensor.matmul(
                            ps[:], wc[:, k, :], x0[:, k, :],
                            start=(k == 0), stop=(k == KC - 1),
                        )
                    nc.scalar.copy(lx[:, m, :], ps[:])
                return xt_sb, h0f, h0b, x0, lx

            def emit_stages(t, h0f, h0b, lx):
                cols = slice(t * TN, (t + 1) * TN)
                # RK4 stages
                ksum = big.tile([128, KC, TN], bf16, tag="ksum")
                h_cur = h0b
                for s in (1, 2, 3, 4):
                    zz = dbl.tile([128, KC, TN], bf16, tag="zz")
                    rr = dbl.tile([128, KC, TN], bf16, tag="rr")
                    for m in range(GM):
                        ps = ppg.tile([128, TN], f32, tag="psg")
                        for k in range(KC):
                            nc.tensor.matmul(
                                ps[:], w_hg_sb[:, k, ts(m, 128)], h_cur[:, k, :],
                                start=(k == 0), stop=(k == KC - 1),
                            )
                        garg = chunk.tile([128, TN], bf16, tag="arg")
                        nc.vector.tensor_add(garg[:], ps[:], lx[:, m, :])
                        dst = zz if m < KC else rr
                        nc.scalar.activation(
                            dst[:, m % KC, :], garg[:], AF.Sigmoid,
                            bias=b_hg_sb[:, m : m + 1],
                        )
                    rh = dbl.tile([128, KC, TN], bf16, tag="rh")
                    for m in range(KC):
                        nc.vector.tensor_mul(rh[:, m, :], rr[:, m, :], h_cur[:, m, :])

                    h_next = (
                        dbl.tile([128, KC, TN], bf16, tag="hn", name="hn", bufs=2)
                        if s < 4 else None
                    )
                    for m in range(KC):
                        ps = ppl.tile([128, TN], f32, tag="psl")
                        for k in range(KC):
                            nc.tensor.matmul(
                                ps[:], w_hl_sb[:, k, ts(m, 128)], rh[:, k, :],
                                start=(k == 0), stop=(k == KC - 1),
                            )
                        larg = chunk.tile([128, TN], bf16, tag="arg")
                        nc.vector.tensor_add(larg[:], ps[:], lx[:, 2 * KC + m, :])
                        tt = chunk.tile([128, TN], bf16, tag="tt")
                        nc.scalar.activation(
                            tt[:], larg[:], AF.Tanh, bias=b_hl_sb[:, m : m + 1]
                        )
                        dd = chunk.tile([128, TN], bf16, tag="dd")
                        nc.vector.tensor_sub(dd[:], tt[:], h_cur[:, m, :])
                        if s == 1:
                            nc.vector.tensor_mul(ksum[:, m, :], zz[:, m, :], dd[:])
                            nc.vector.scalar_tensor_tensor(
                                h_next[:, m, :], ksum[:, m, :], dt_val / 2,
                                h0b[:, m, :], ALU.mult, ALU.add,
                            )
                        else:
                            kk = chunk.tile([128, TN], bf16, tag="kk")
                            nc.vector.tensor_mul(kk[:], zz[:, m, :], dd[:])
                            w_s = {2: 2.0, 3: 2.0, 4: 1.0}[s]
                            nc.vector.scalar_tensor_tensor(
                                ksum[:, m, :], kk[:], w_s, ksum[:, m, :],
                                ALU.mult, ALU.add,
                            )
                            if s < 4:
                                c_s = dt_val / 2 if s == 2 else dt_val
                                nc.vector.scalar_tensor_tensor(
                                    h_next[:, m, :], kk[:], c_s, h0b[:, m, :],
                                    ALU.mult, ALU.add,
                                )
                    if s < 4:
                        h_cur = h_next

                # state_new = h0 + dt/6 * ksum  (fp32, in place over h0f)
                snb = dbl.tile([128, KC, TN], bf16, tag="x0", bufs=2, name="snb")
                for m in range(KC):
                    nc.vector.scalar_tensor_tensor(
                        h0f[:, m, :], ksum[:, m, :], dt_val / 6, h0f[:, m, :],
                        ALU.mult, ALU.add,
                    )
                    nc.vector.tensor_copy(snb[:, m, :], h0f[:, m, :])
                dma(snt[:, :, cols], h0f[:])
                return snb

            def emit_y(t, snb):
                cols = slice(t * TN, (t + 1) * TN)
                # y = W_y2 @ tanh(W_y1 @ state_new)
                gy = dbl.tile([128, KC, TN], bf16, tag="zz", name="gy")
                for m in range(KC):
                    wc = wstream.tile([128, KC, 128], bf16, tag="wst")
                    dma(wc[:], w_y1[:, :, ts(m, 128)])
                    ps = pp1.tile([128, TN], f32, tag="ps1")
                    for k in range(KC):
                        nc.tensor.matmul(
                            ps[:], wc[:, k, :], snb[:, k, :],
                            start=(k == 0), stop=(k == KC - 1),
                        )
                    nc.scalar.activation(gy[:, m, :], ps[:], AF.Tanh)
                y_sb = dbl.tile([128, YM, TN], f32, tag="y")
                for m in range(YM):
                    wc = wstream.tile([128, KC, 128], bf16, tag="wst")
                    dma(wc[:], w_y2[:, :, ts(m, 128)])
                    ps = pp1.tile([128, TN], f32, tag="ps1")
                    for k in range(KC):
                        nc.tensor.matmul(
                            ps[:], wc[:, k, :], gy[:, k, :],
                            start=(k == 0), stop=(k == KC - 1),
                        )
                    nc.scalar.copy(y_sb[:, m, :], ps[:])
                dma(yt[:, :, cols], y_sb[:])

            head = emit_head(0)
            for t in range(NT):
                _, h0f, h0b, _, lx = head
                snb = emit_stages(t, h0f, h0b, lx)
                if t + 1 < NT:
                    head = emit_head(t + 1)
                emit_y(t, snb)

    nc.compile()
    return nc


def _get_nc(dt_val: float):
    key = round(float(dt_val), 9)
    if key not in _cache:
        _cache[key] = _build(key)
    return _cache[key]


def kernel(x, state, dt, W_in, b_in, W_lx, W_hg, b_hg, W_hl, b_hl, W_y1, W_y2,
           trace=False):
    from concourse.bass_utils import run_bass_kernel_spmd

    dt_val = float(np.asarray(dt).reshape(-1)[0])
    nc = _get_nc(dt_val)

    x = np.asarray(x, np.float32)
    state = np.asarray(state, np.float32)
    weights = {
        "w_in": _pack_w(np.asarray(W_in, np.float32), KIC),
        "w_lx": _pack_w(np.asarray(W_lx, np.float32), KC),
        "w_hg": _pack_w(np.asarray(W_hg, np.float32), KC),
        "w_hl": _pack_w(np.asarray(W_hl, np.float32), KC),
        "w_y1": _pack_w(np.asarray(W_y1, np.float32), KC),
        "w_y2": _pack_w(np.asarray(W_y2, np.float32), KC),
        "b_in": _pack_b(np.asarray(b_in, np.float32)),
        "b_hg": _pack_b(np.asarray(b_hg, np.float32)),
        "b_hl": _pack_b(np.asarray(b_hl, np.float32)),
    }
    in_maps = []
    for c in range(NCORES):
        rows = slice(c * BC, (c + 1) * BC)
        m = dict(weights)
        m["xt"] = _pack_act(x[rows], KIC, _BF16)
        m["st"] = _pack_act(state[rows], KC, np.float32)
        in_maps.append(m)

    res = run_bass_kernel_spmd(nc, in_maps, core_ids=list(range(NCORES)),
                               trace=trace)

    y = np.empty((B, KO), np.float32)
    sn = np.empty((B, K), np.float32)
    for c in range(NCORES):
        rows = slice(c * BC, (c + 1) * BC)
        y[rows] = _unpack_act(res.results[c]["yt"], YM, BC)
        sn[rows] = _unpack_act(res.results[c]["snt"], KC, BC)
    if trace:
        kernel.last_results = res
    return y, sn


# revision 7
# speedup vs baseline: 1.1133x; 1.0443x over previous
"""HOGRUCell (RK4-integrated GRU-like ODE cell) Trainium2 Bass kernel.

Strategy: pure data parallel over 8 NeuronCores (batch sharded). All compute
in a transposed [features, batch] layout so every linear-layer contraction is
already on the partition dim and biases become per-partition ACT scalars.
Matmuls in bf16 (PE-bound problem: ~72 GFLOP/core), state updates fp32 at the
final combine.
"""

import numpy as np
import ml_dtypes

B, KI, K, KO = 16384, 256, 1024, 256
NCORES = 8
BC = B // NCORES          # 2048 batch rows per core
TN = 512                  # batch tile (free dim per matmul)
NT = BC // TN             # 4 tiles per core
KC = K // 128             # 8 chunks of the 1024 state dim
KIC = KI // 128           # 2 chunks of input dim
GM = 2 * K // 128         # 16 chunks of the 2048 gate dim
LXM = 3 * K // 128        # 24 chunks of the 3072 lx dim
YM = KO // 128            # 2 chunks of output dim

_BF16 = ml_dtypes.bfloat16

_cache = {}


def _pack_w(w, kc):
    """[out_f, in_f] weight -> lhsT packed [128, kc, out_f] (k-chunk on free)."""
    in_f, out_f = w.shape[1], w.shape[0]
    assert in_f == kc * 128
    return np.ascontiguousarray(
        w.T.reshape(kc, 128, out_f).transpose(1, 0, 2)
    ).astype(_BF16)


def _pack_b(b):
    """[F] bias -> [128, F//128] per-partition scalars."""
    return np.ascontiguousarray(b.reshape(-1, 128).T).astype(np.float32)


def _pack_act(a, kc, dtype):
    """[rows, F] activations -> [128, kc, rows] transposed+chunked."""
    rows = a.shape[0]
    return np.ascontiguousarray(
        a.T.reshape(kc, 128, rows).transpose(1, 0, 2)
    ).astype(dtype)


def _unpack_act(a, kc, rows):
    """[128, kc, rows] -> [rows, kc*128]."""
    return np.ascontiguousarray(a.transpose(1, 0, 2).reshape(kc * 128, rows).T)


def _build(dt_val: float):
    import concourse.bacc as bacc
    import concourse.mybir as mybir
    import concourse.tile as tile
    from concourse.bass import ts

    f32 = mybir.dt.float32
    bf16 = mybir.dt.bfloat16
    AF = mybir.ActivationFunctionType
    ALU = mybir.AluOpType

    nc = bacc.Bacc("TRN2", target_bir_lowering=False, debug=False)

    xt = nc.dram_tensor("xt", [128, KIC, BC], bf16, kind="ExternalInput")
    st = nc.dram_tensor("st", [128, KC, BC], f32, kind="ExternalInput")
    w_in = nc.dram_tensor("w_in", [128, KIC, K], bf16, kind="ExternalInput")
    w_lx = nc.dram_tensor("w_lx", [128, KC, 3 * K], bf16, kind="ExternalInput")
    w_hg = nc.dram_tensor("w_hg", [128, KC, 2 * K], bf16, kind="ExternalInput")
    w_hl = nc.dram_tensor("w_hl", [128, KC, K], bf16, kind="ExternalInput")
    w_y1 = nc.dram_tensor("w_y1", [128, KC, K], bf16, kind="ExternalInput")
    w_y2 = nc.dram_tensor("w_y2", [128, KC, KO], bf16, kind="ExternalInput")
    b_in = nc.dram_tensor("b_in", [128, KC], f32, kind="ExternalInput")
    b_hg = nc.dram_tensor("b_hg", [128, GM], f32, kind="ExternalInput")
    b_hl = nc.dram_tensor("b_hl", [128, KC], f32, kind="ExternalInput")
    yt = nc.dram_tensor("yt", [128, YM, BC], f32, kind="ExternalOutput")
    snt = nc.dram_tensor("snt", [128, KC, BC], f32, kind="ExternalOutput")

    dma = nc.sync.dma_start

    with tile.TileContext(nc) as tc:
        with (
            tc.tile_pool(name="wres", bufs=1) as wres,
            tc.tile_pool(name="wstream", bufs=4) as wstream,
            tc.tile_pool(name="big", bufs=1) as big,
            tc.tile_pool(name="dbl", bufs=1) as dbl,
            tc.tile_pool(name="chunk", bufs=2) as chunk,
            tc.tile_pool(name="ppg", bufs=3, space="PSUM") as ppg,
            tc.tile_pool(name="ppl", bufs=2, space="PSUM") as ppl,
            tc.tile_pool(name="pp1", bufs=2, space="PSUM") as pp1,
        ):
            # resident weights
            w_in_sb = wres.tile([128, KIC, K], bf16)
            dma(w_in_sb[:], w_in[:])
            w_hg_sb = wres.tile([128, KC, 2 * K], bf16)
            w_hl_sb = wres.tile([128, KC, K], bf16)
            b_in_sb = wres.tile([128, KC], f32)
            dma(b_in_sb[:], b_in[:])
            b_hg_sb = wres.tile([128, GM], f32)
            dma(b_hg_sb[:], b_hg[:])
            b_hl_sb = wres.tile([128, KC], f32)
            dma(b_hl_sb[:], b_hl[:])

            def emit_head(t):
                cols = slice(t * TN, (t + 1) * TN)
                xt_sb = dbl.tile([128, KIC, TN], bf16, tag="xt", name="xt_sb")
                nc.scalar.dma_start(xt_sb[:], xt[:, :, cols])
                h0f = dbl.tile([128, KC, TN], f32, tag="h0f", name="h0f")
                nc.scalar.dma_start(h0f[:], st[:, :, cols])
                h0b = dbl.tile([128, KC, TN], bf16, tag="h0b", name="h0b")
                for m in range(KC):
                    nc.vector.tensor_copy(h0b[:, m, :], h0f[:, m, :])

                # x0 = tanh(W_in @ x + b_in)
                x0 = dbl.tile([128, KC, TN], bf16, tag="x0", bufs=2, name="x0")
                for m in range(KC):
                    ps = pp1.tile([128, TN], f32, tag="ps1")
                    for k in range(KIC):
                        nc.tensor.matmul(
                            ps[:], w_in_sb[:, k, ts(m, 128)], xt_sb[:, k, :],
                            start=(k == 0), stop=(k == KIC - 1),
                        )
                    nc.scalar.activation(
                        x0[:, m, :], ps[:], AF.Tanh, bias=b_in_sb[:, m : m + 1]
                    )

                # lx = W_lx @ x0   (streamed weights, bf16 copy to SBUF)
                lx = big.tile([128, LXM, TN], bf16, tag="lx", name="lx")
                for m in range(LXM):
                    wc = wstream.tile([128, KC, 128], bf16, tag="wst")
                    dma(wc[:], w_lx[:, :, ts(m, 128)])
                    ps = pp1.tile([128, TN], f32, tag="ps1")
                    for k in range(KC):
                        nc.tensor.matmul(
                            ps[:], wc[:, k, :], x0[:, k, :],
                            start=(k == 0), stop=(k == KC - 1),
                        )
                    nc.scalar.copy(lx[:, m, :], ps[:])
                return xt_sb, h0f, h0b, x0, lx

            def emit_stages(t, h0f, h0b, lx, filler):
                cols = slice(t * TN, (t + 1) * TN)
                # RK4 stages
                ksum = big.tile([128, KC, TN], bf16, tag="ksum")
                h_cur = h0b
                for s in (1, 2, 3, 4):
                    zz = dbl.tile([128, KC, TN], bf16, tag="zz")
                    rr = dbl.tile([128, KC, TN], bf16, tag="rr")
                    for m in range(GM):
                        ps = ppg.tile([128, TN], f32, tag="psg")
                        for k in range(KC):
                            nc.tensor.matmul(
                                ps[:], w_hg_sb[:, k, ts(m, 128)], h_cur[:, k, :],
                                start=(k == 0), stop=(k == KC - 1),
                            )
                        garg = chunk.tile([128, TN], bf16, tag="arg")
                        nc.vector.tensor_add(garg[:], ps[:], lx[:, m, :])
                        dst = zz if m < KC else rr
                        nc.scalar.activation(
                            dst[:, m % KC, :], garg[:], AF.Sigmoid,
                            bias=b_hg_sb[:, m : m + 1],
                        )
                    rh = dbl.tile([128, KC, TN], bf16, tag="rh")
                    for m in range(KC):
                        nc.vector.tensor_mul(rh[:, m, :], rr[:, m, :], h_cur[:, m, :])

                    h_next = (
                        dbl.tile([128, KC, TN], bf16, tag="hn", name="hn", bufs=2)
                        if s < 4 else None
                    )
                    for m in range(KC):
                        ps = ppl.tile([128, TN], f32, tag="psl")
                        for k in range(KC):
                            nc.tensor.matmul(
                                ps[:], w_hl_sb[:, k, ts(m, 128)], rh[:, k, :],
                                start=(k == 0), stop=(k == KC - 1),
                            )
                        larg = chunk.tile([128, TN], bf16, tag="arg")
                        nc.vector.tensor_add(larg[:], ps[:], lx[:, 2 * KC + m, :])
                        tt = chunk.tile([128, TN], bf16, tag="tt")
                        nc.scalar.activation(
                            tt[:], larg[:], AF.Tanh, bias=b_hl_sb[:, m : m + 1]
                        )
                        dd = chunk.tile([128, TN], bf16, tag="dd")
                        nc.vector.tensor_sub(dd[:], tt[:], h_cur[:, m, :])
                        if s == 1:
                            nc.vector.tensor_mul(ksum[:, m, :], zz[:, m, :], dd[:])
                            nc.vector.scalar_tensor_tensor(
                                h_next[:, m, :], ksum[:, m, :], dt_val / 2,
                                h0b[:, m, :], ALU.mult, ALU.add,
                            )
                        else:
                            kk = chunk.tile([128, TN], bf16, tag="kk")
                            nc.vector.tensor_mul(kk[:], zz[:, m, :], dd[:])
                            w_s = {2: 2.0, 3: 2.0, 4: 1.0}[s]
                            nc.vector.scalar_tensor_tensor(
                                ksum[:, m, :], kk[:], w_s, ksum[:, m, :],
                                ALU.mult, ALU.add,
                            )
                            if s < 4:
                                c_s = dt_val / 2 if s == 2 else dt_val
                                nc.vector.scalar_tensor_tensor(
                                    h_next[:, m, :], kk[:], c_s, h0b[:, m, :],
                                    ALU.mult, ALU.add,
                                )
                    if s < 4:
                        h_cur = h_next
                        filler()

                # state_new = h0 + dt/6 * ksum  (fp32, in place over h0f)
                snb = dbl.tile([128, KC, TN], bf16, tag="x0", bufs=2, name="snb")
                for m in range(KC):
                    nc.vector.scalar_tensor_tensor(
                        h0f[:, m, :], ksum[:, m, :], dt_val / 6, h0f[:, m, :],
                        ALU.mult, ALU.add,
                    )
                    nc.vector.tensor_copy(snb[:, m, :], h0f[:, m, :])
                nc.scalar.dma_start(snt[:, :, cols], h0f[:])
                return snb

            def make_y_thunks(t, snb):
                cols = slice(t * TN, (t + 1) * TN)
                gy = dbl.tile([128, KC, TN], bf16, tag="gy", name="gy")
                y_sb = dbl.tile([128, YM, TN], f32, tag="y", name="y_sb")

                def y1_part(ms):
                    for m in ms:
                        wc = wstream.tile([128, KC, 128], bf16, tag="wst",
                                          name="wc")
                        dma(wc[:], w_y1[:, :, ts(m, 128)])
                        ps = pp1.tile([128, TN], f32, tag="ps1", name="ps")
                        for k in range(KC):
                            nc.tensor.matmul(
                                ps[:], wc[:, k, :], snb[:, k, :],
                                start=(k == 0), stop=(k == KC - 1),
                            )
                        nc.scalar.activation(gy[:, m, :], ps[:], AF.Tanh)

                def y2_part():
                    for m in range(YM):
                        wc = wstream.tile([128, KC, 128], bf16, tag="wst",
                                          name="wc")
                        dma(wc[:], w_y2[:, :, ts(m, 128)])
                        ps = pp1.tile([128, TN], f32, tag="ps1", name="ps")
                        for k in range(KC):
                            nc.tensor.matmul(
                                ps[:], wc[:, k, :], gy[:, k, :],
                                start=(k == 0), stop=(k == KC - 1),
                            )
                        nc.scalar.copy(y_sb[:, m, :], ps[:])
                    nc.scalar.dma_start(yt[:, :, cols], y_sb[:])

                return [
                    lambda: y1_part(range(0, 3)),
                    lambda: y1_part(range(3, 6)),
                    lambda: (y1_part(range(6, KC)), y2_part()),
                ]

            pending = []

            def filler():
                if pending:
                    pending.pop(0)()

            head = emit_head(0)
            dma(w_hg_sb[:], w_hg[:])
            dma(w_hl_sb[:], w_hl[:])
            for t in range(NT):
                _, h0f, h0b, _, lx = head
                snb = emit_stages(t, h0f, h0b, lx, filler)
                assert not pending
                if t + 1 < NT:
                    head = emit_head(t + 1)
                pending = make_y_thunks(t, snb)
            for th in pending:
                th()

    nc.compile()
    return nc


def _get_nc(dt_val: float):
    key = round(float(dt_val), 9)
    if key not in _cache:
        _cache[key] = _build(key)
    return _cache[key]


def kernel(x, state, dt, W_in, b_in, W_lx, W_hg, b_hg, W_hl, b_hl, W_y1, W_y2,
           trace=False):
    from concourse.bass_utils import run_bass_kernel_spmd

    dt_val = float(np.asarray(dt).reshape(-1)[0])
    nc = _get_nc(dt_val)

    x = np.asarray(x, np.float32)
    state = np.asarray(state, np.float32)
    weights = {
        "w_in": _pack_w(np.asarray(W_in, np.float32), KIC),
        "w_lx": _pack_w(np.asarray(W_lx, np.float32), KC),
        "w_hg": _pack_w(np.asarray(W_hg, np.float32), KC),
        "w_hl": _pack_w(np.asarray(W_hl, np.float32), KC),
        "w_y1": _pack_w(np.asarray(W_y1, np.float32), KC),
        "w_y2": _pack_w(np.asarray(W_y2, np.float32), KC),
        "b_in": _pack_b(np.asarray(b_in, np.float32)),
        "b_hg": _pack_b(np.asarray(b_hg, np.float32)),
        "b_hl": _pack_b(np.asarray(b_hl, np.float32)),
    }
    in_maps = []
    for c in range(NCORES):
        rows = slice(c * BC, (c + 1) * BC)
        m = dict(weights)
        m["xt"] = _pack_act(x[rows], KIC, _BF16)
        m["st"] = _pack_act(state[rows], KC, np.float32)
        in_maps.append(m)

    res = run_bass_kernel_spmd(nc, in_maps, core_ids=list(range(NCORES)),
                               trace=trace)

    y = np.empty((B, KO), np.float32)
    sn = np.empty((B, K), np.float32)
    for c in range(NCORES):
        rows = slice(c * BC, (c + 1) * BC)
        y[rows] = _unpack_act(res.results[c]["yt"], YM, BC)
        sn[rows] = _unpack_act(res.results[c]["snt"], KC, BC)
    if trace:
        kernel.last_results = res
    return y, sn
